# revision 6
# baseline (speedup 1.0000x reference)
"""Trainium2 Bass kernel for nn_BTGRule — per-core exact-shape design (v3).

Reference computation:
    L = span_rep @ Wl + bl            # [65, 65, 512]
    R = span_rep @ Wr + br            # [65, 65, 512]
    H = tanh(L[i, j] + R[j, k])       # over valid triples i < j < k
    scores[i, j, k] = H @ Wout + bout # [65, 65, 65, 2]

v3 drops the SPMD single-program constraint: each core gets its own
compiled program with EXACT block shapes (j-blocks (a, w), a = min(j,
64-j), w = 64-a, transposed for j > 32), so the ~20% slot padding of the
v2 design disappears.  Cores are assigned 8 j's each (63 total) with LPT
balancing of the H-column count.

Per-core pipeline (per rep):
  PE : span projections per hout-tile (Wl/Wr groups, chunked psum), then
       score matmuls of the PREVIOUS rep (software pipelining).
  DVE: proj PSUM->SBUF f16 copies, fused broadcast-add L+R per (h-tile,
       block) via tensor_tensor with duplicated-pair APs (2x mode), and
       2/3 of the score copies.
  ACT: tanh per h-tile (4 big instrs, FD ~5500) with the (bl+br) bias
       applied FREE via the activation bias operand; 1/3 of score copies.
  H layout is h-tile-major so each tanh covers one h-tile => per-
       partition bias is exact.
  Scores are f16 (well within tolerance), host adds bout + upcasts.
"""

import numpy as np

N1 = 65
HID = 512
HT = 4            # 128-row h tiles
OUT = 2
NCORES = 8


# ---------------------------------------------------------------------------
# Block assignment: j in [1, 63], area j*(64-j); LPT onto 8 cores (<=8 each)
# ---------------------------------------------------------------------------

# Precomputed swap-optimized partition of j=1..63: per-core padded H-column
# counts are 5522-5526 (ideal 5460 with odd-w padding included).
CORE_JS = [
    [3, 8, 18, 25, 30, 31, 38],
    [1, 2, 19, 28, 35, 40, 49, 50],
    [4, 12, 20, 33, 36, 51, 53, 56],
    [13, 17, 21, 39, 42, 45, 61, 62],
    [5, 26, 32, 41, 44, 46, 59, 60],
    [6, 15, 16, 27, 34, 43, 52, 63],
    [11, 14, 29, 37, 48, 54, 55, 58],
    [7, 9, 10, 22, 23, 24, 47, 57],
]


class Block:
    def __init__(self, j):
        self.j = j
        self.trans = j > 32
        self.a = j if not self.trans else 64 - j
        self.w = 64 - self.a
        self.wpad = self.w + (self.w % 2)          # even dense width
        self.area = self.a * self.wpad
        # offsets filled by CoreLayout
        self.boff = None          # H col offset within an h-tile section
        self.dsec = None          # 'l' or 'r': dense spans weight group
        self.doff = None          # dense offset within its D section
        self.coff = None          # col offset (units of spans) in C section


class CoreLayout:
    """Span/H/score layout for one core's block list."""

    def __init__(self, js, build_n=0):
        self.blocks = [Block(j) for j in js]
        # built blocks (PE indicator-matmul path): largest with area <= 1024,
        # placed LAST in the H layout so the SBUF tanh covers a prefix
        cand = sorted((b for b in self.blocks if b.area <= 1024),
                      key=lambda b: -b.area)
        self.built = cand[:build_n]
        self.blocks = ([b for b in self.blocks if b not in self.built]
                       + self.built)
        self.nonbuilt = [b for b in self.blocks if b not in self.built]
        # H layout (within one h-tile section), block order as given
        off = 0
        for b in self.blocks:
            b.boff = off
            off += b.area
        self.S_nb = self.built[0].boff if self.built else off
        self.barea = off - self.S_nb
        # group span layout for built blocks: [64 L | 64 R] per block
        # (normal: col side = L spans, dense = R; transposed swapped)
        self.g_rows = 128 * len(self.built)   # padded rows
        gl = []                                # (block, side) metadata
        for gb, b in enumerate(self.built):
            gl.append((gb * 128, gb * 128 + 64))
        self.g_off = gl
        self.S = off                                   # h-tile section cols
        # span sections: [WlD | WlC | WrD | WrC]
        # normal: col side=L(Wl dup), dense=R(Wr); transposed: col=R, dense=L
        wld = wlc = wrd = wrc = 0
        for b in self.nonbuilt:
            if b.trans:
                b.dsec = 'l'
                b.doff = wld
                wld += b.wpad
                b.coff = wrc
                wrc += b.a
            else:
                b.dsec = 'r'
                b.doff = wrd
                wrd += b.wpad
                b.coff = wlc
                wlc += b.a
        self.WlD, self.WlC, self.WrD, self.WrC = wld, 2 * wlc, wrd, 2 * wrc
        self.o_wld = 0
        self.o_wlc = self.o_wld + self.WlD
        self.o_wrd = self.o_wlc + self.WlC
        self.o_wrc = self.o_wrd + self.WrD
        self.SPANCOLS = self.o_wrc + self.WrC
        assert self.SPANCOLS % 2 == 0
        # proj psum chunks (within Wl region [0, o_wrd) and Wr region)
        self.wl_cols = self.WlD + self.WlC
        self.wr_cols = self.WrD + self.WrC
        self.pchunks = []          # (offset, width, wgrp)
        for base, cols, grp in ((0, self.wl_cols, 'l'),
                                (self.o_wrd, self.wr_cols, 'r')):
            n = -(-cols // 512)
            step = -(-cols // (2 * n)) * 2
            c = 0
            while c < cols:
                w = min(step, cols - c)
                self.pchunks.append((base + c, w, grp))
                c += w
        # score chunks over [0, S); uniform width, the last chunk slides
        # back (overlap recomputes a few columns — harmless) so every chunk
        # is the same width for the partition-packed psum layout
        n = -(-self.S // (426 if build_n else 512))
        step = -(-self.S // (2 * n)) * 2
        self.schunks = [(min(i * step, self.S - step), step)
                        for i in range(n)]
        # packed-score groups: 3 chunks -> one [96, step] psum tile at
        # partition offsets 0/32/64 (the AP layer rejects offset 96)
        self.sgroups = [list(range(g, min(g + 3, n)))
                        for g in range(0, n, 3)]
        self.pg_off = [gi * step for gi in range(len(self.sgroups))]
        self.PW = len(self.sgroups) * step
        self.sstep = step

    def dense_off(self, b):        # abs span-col offset of block's dense run
        return (self.o_wld if b.dsec == 'l' else self.o_wrd) + b.doff

    def col_off(self, b):          # abs span-col offset of block's dup cols
        return (self.o_wlc if b.dsec == 'r' else self.o_wrc) + 2 * b.coff


BUILD_N = 0        # blocks per core on the PE indicator-build path
LAYOUTS = [CoreLayout(js, BUILD_N) for js in CORE_JS]


def set_build_n(n):
    global BUILD_N, LAYOUTS
    BUILD_N = n
    LAYOUTS = [CoreLayout(js, n) for js in CORE_JS]
    _COMPILED.clear()


# ---------------------------------------------------------------------------
# Program builder (per core)
# ---------------------------------------------------------------------------

_COMPILED = {}

# schedule knobs (tuned via TimelineSim)
CFG = {
    "ps_pr_bufs": 2,     # proj psum pool buffers
    "ps_sc_bufs": 6,     # score psum pool buffers
    "sc_mode": ["post", "end", "end", "end",
                "end", "end", "end", "end"],
    "act_step": 99,      # packed copies are cheap: keep all on DVE
    "act_pc": 0,         # number of proj copies (of 8) done on ACT
    "sc_pair": 1,        # score chunks per psum tile
    "tanh_merge": 1,     # h-tile sections per tanh instr (1: bias via ACT;
                         # >1: bias added in projections via ones-matmul)
    "wp_q": "gpsimd",    # wp (weights) DMA issued from the idle Pool queue
    "sc_pack": True,     # pack 4 score chunks per psum tile at partition
                         # offsets 0/32/64/96 so one copy evacuates all 4
}


def _cfg(key, core):
    v = CFG[key]
    return v[core] if isinstance(v, (list, tuple)) else v


def _build_program(core, reps=1, unroll=False, inner=1):
    import contextlib

    import concourse.bacc as bacc
    import concourse.mybir as mybir
    import concourse.tile as tile

    lay = LAYOUTS[core]
    f32 = mybir.dt.float32
    f16 = mybir.dt.float16
    tanh = mybir.ActivationFunctionType.Tanh
    ident = mybir.ActivationFunctionType.Identity
    add = mybir.AluOpType.add

    SC, S = lay.SPANCOLS, lay.S

    nc = bacc.Bacc("TRN2", target_bir_lowering=False, debug=False,
                   num_devices=1)

    wp_d = nc.declare_dram_parameter("wp", [128, 2 * HT * HID], f16,
                                     isOutput=False)
    sp_d = nc.declare_dram_parameter("sp", [128, HT * SC], f16,
                                     isOutput=False)
    wout_d = nc.declare_dram_parameter("wout", [128, HT * OUT], f16,
                                       isOutput=False)
    blbr_d = nc.declare_dram_parameter("blbr", [128, HT], f32,
                                       isOutput=False)
    MERGE = _cfg("tanh_merge", core)
    if MERGE > 1:
        blbr16_d = nc.declare_dram_parameter("blbr16", [1, HID], f16,
                                             isOutput=False)
    if lay.built:
        spg_d = nc.declare_dram_parameter("spg", [128, HT * lay.g_rows],
                                          f16, isOutput=False)
        ind_d = nc.declare_dram_parameter("ind", [128, lay.barea], f16,
                                          isOutput=False)
    PACK = _cfg("sc_pack", core)
    if PACK:
        out_d = nc.declare_dram_parameter("out", [96, lay.PW], f16,
                                          isOutput=True)
        wout32_d = nc.declare_dram_parameter("wout32", [128, HT * 32], f16,
                                             isOutput=False)
    else:
        out_d = nc.declare_dram_parameter("out", [OUT, S], f16,
                                          isOutput=True)

    with tile.TileContext(nc) as tc:
        with (
            tc.tile_pool(name="stream", bufs=2) as spool,
            tc.tile_pool(name="hbuf", bufs=2) as hpool,
            tc.tile_pool(name="obuf", bufs=2) as opool,
            tc.tile_pool(name="ps_pr", bufs=CFG["ps_pr_bufs"],
                         space="PSUM") as ps_pr,
            tc.tile_pool(name="ps_sc", bufs=CFG["ps_sc_bufs"],
                         space="PSUM") as ps_sc,
            tc.tile_pool(name="ps_bd", bufs=1, space="PSUM") as ps_bd,
            tc.For_i(0, reps // inner, 1,
                     hint_engines=(mybir.EngineType.PE,
                                   mybir.EngineType.DVE,
                                   mybir.EngineType.Activation,
                                   mybir.EngineType.SP))
            if reps > inner and not unroll else contextlib.nullcontext(),
        ):
            prev = [None]          # (H_t, wout_t) of the previous rep
            NS = len(lay.schunks)

            # score groups: schunks paired (sc_pair) into one psum tile so a
            # single evacuation copy covers the pair
            PAIR = CFG.get("sc_pair", 1)
            SGROUPS = [lay.schunks[g:g + PAIR]
                       for g in range(0, NS, PAIR)]
            NG = len(SGROUPS)
            if PACK:
                # partition-packed grouping: pad short groups by repeating
                # the last chunk (recomputed, harmless) so all 128 psum
                # rows are written
                PGRP = [(g + [g[-1]] * (3 - len(g)))
                        for g in lay.sgroups]
                NG = len(PGRP)
            SLOT_GROUPS = [list(range((NG * t) // HT, (NG * (t + 1)) // HT))
                           for t in range(HT)]
            ACT_GROUPS = set(range(CFG["act_step"] - 1, NG, CFG["act_step"]))

            def emit_score_mms(st, gis):
                H_t, wout_t, w32 = st
                out = []
                if PACK:
                    for gi in gis:
                        psc = ps_sc.tile([96, 512], f32, tag="pssc")
                        for k, ci in enumerate(PGRP[gi]):
                            c0, cw = lay.schunks[ci]
                            for t in range(HT):
                                nc.tensor.matmul(
                                    psc[32 * k:32 * k + 32, 0:cw],
                                    w32[:, 32 * t:32 * (t + 1)],
                                    H_t[:, t * S + c0:t * S + c0 + cw],
                                    start=(t == 0), stop=(t == HT - 1))
                        out.append((gi, lay.pg_off[gi], lay.sstep,
                                    psc[:, 0:lay.sstep]))
                    return out
                for gi in gis:
                    chunks = SGROUPS[gi]
                    g0 = chunks[0][0]
                    gw = sum(cw for _, cw in chunks)
                    psc = ps_sc.tile([OUT, gw], f32, tag="pssc")
                    for (c0, cw) in chunks:
                        for t in range(HT):
                            nc.tensor.matmul(
                                psc[:, c0 - g0:c0 - g0 + cw],
                                wout_t[:, OUT * t:OUT * (t + 1)],
                                H_t[:, t * S + c0:t * S + c0 + cw],
                                start=(t == 0), stop=(t == HT - 1))
                    out.append((gi, g0, gw, psc))
                return out

            def emit_score_copies(out_sb, pend):
                for gi, g0, gw, psc in pend:
                    dst = out_sb[:, g0:g0 + gw]
                    if gi in ACT_GROUPS:
                        nc.scalar.activation(dst, psc[:], ident)
                    else:
                        nc.vector.tensor_copy(dst, psc[:])
                if pend:
                    g0 = pend[0][1]
                    g1, gw1 = pend[-1][1], pend[-1][2]
                    nc.sync.dma_start(out_d[:, g0:g1 + gw1],
                                      out_sb[:, g0:g1 + gw1])

            for _rep in range(reps if unroll else inner):
                sp_t = spool.tile([128, HT * SC], f16, tag="sp")
                wp_t = spool.tile([128, 2 * HT * HID], f16, tag="wp")
                wout_t = spool.tile([128, HT * OUT], f16, tag="wout")
                blbr_t = spool.tile([128, HT], f32, tag="blbr")
                wq = (nc.gpsimd if _cfg("wp_q", core) == "gpsimd"
                      else nc.scalar)
                nc.sync.dma_start(sp_t[:, 0:2 * SC], sp_d[:, 0:2 * SC])
                wq.dma_start(wp_t[:, 0:2048], wp_d[:, 0:2048])
                nc.sync.dma_start(sp_t[:, 2 * SC:], sp_d[:, 2 * SC:])
                wq.dma_start(wp_t[:, 2048:], wp_d[:, 2048:])
                nc.scalar.dma_start(wout_t[:], wout_d[:])
                nc.scalar.dma_start(blbr_t[:], blbr_d[:])
                if PACK:
                    wout32_t = spool.tile([128, HT * 32], f16, tag="wout32")
                    nc.scalar.dma_start(wout32_t[:], wout32_d[:])
                if MERGE > 1:
                    blbr16_t = spool.tile([1, HID], f16, tag="blbr16")
                    nc.scalar.dma_start(blbr16_t[:], blbr16_d[:])
                    ones_t = spool.tile([1, 512], f16, tag="ones")
                    nc.vector.memset(ones_t[:], 1.0)
                if lay.built:
                    spg_t = spool.tile([128, HT * lay.g_rows], f16,
                                       tag="spg")
                    ind_t = spool.tile([128, lay.barea], f16, tag="ind")
                    nc.gpsimd.dma_start(spg_t[:], spg_d[:])
                    nc.gpsimd.dma_start(ind_t[:], ind_d[:])

                # weight block: wgrp l/r, hout t, hin hi; layout is
                # (kind, hi)-major so all hout tiles of one (kind, hi) are
                # contiguous (used as a single 512-wide rhs by projT)
                def wblk(grp, t, hi):
                    kind = 0 if grp == 'l' else 1
                    c0 = (kind * HT + hi) * HID + t * 128
                    return wp_t[:, c0:c0 + 128]

                proj_sb = spool.tile([128, HT * SC], f16, tag="proj")
                H_t = hpool.tile([128, HT * S], f16, tag="H")
                if prev[0] is not None:
                    out_sb = opool.tile([96, lay.PW] if PACK else [OUT, S],
                                        f16, tag="osb")
                else:
                    out_sb = None
                pend = []

                # transposed projections for the built blocks' span group:
                # projT[span, h] = spg.T @ W, both weight halves, all houts
                if lay.built:
                    pt = ps_pr.tile([128, HID], f32, tag="pspr")
                    projT_sb = spool.tile([128, HID], f16, tag="projT")
                    for lo, grp in ((0, 'l'), (64, 'r')):
                        for hi in range(HT):
                            rhs0 = (0 if grp == 'l' else HT) * HID \
                                + hi * HID
                            nc.tensor.matmul(
                                pt[lo:lo + 64, :],
                                spg_t[:, hi * lay.g_rows + lo:
                                      hi * lay.g_rows + lo + 64],
                                wp_t[:, rhs0:rhs0 + HID],
                                start=(hi == 0), stop=(hi == HT - 1))
                    nc.vector.tensor_copy(projT_sb[:], pt[:])

                pc_cnt = [0]
                for t in range(HT):
                    if _cfg("sc_mode", core) == "pre" and pend:
                        emit_score_copies(out_sb, pend)
                        pend = []
                    # projections for hout-tile t (PE) + copies (DVE/ACT)
                    for (c0, cw, grp) in lay.pchunks:
                        ps = ps_pr.tile([128, cw], f32, tag="pspr")
                        if MERGE > 1:
                            # split chunk into D / C parts; (bl+br) is folded
                            # into the C (duplicated-column) operands here so
                            # merged tanh instrs need no per-partition bias
                            crange = ((lay.o_wlc, lay.o_wrd) if grp == 'l'
                                      else (lay.o_wrc, lay.SPANCOLS))
                            cv0 = max(c0, crange[0])
                            cv1 = min(c0 + cw, crange[1])
                            parts = []
                            if cv0 > c0:
                                parts.append((c0, cv0, False))
                            if cv1 > cv0:
                                parts.append((cv0, cv1, True))
                            for (p0, p1, isc) in parts:
                                for hi in range(HT):
                                    nc.tensor.matmul(
                                        ps[:, p0 - c0:p1 - c0],
                                        wblk(grp, t, hi),
                                        sp_t[:, hi * SC + p0:hi * SC + p1],
                                        start=(hi == 0),
                                        stop=(hi == HT - 1) and not isc)
                                if isc:
                                    nc.tensor.matmul(
                                        ps[:, p0 - c0:p1 - c0],
                                        blbr16_t[0:1,
                                                 t * 128:(t + 1) * 128],
                                        ones_t[0:1, 0:p1 - p0],
                                        start=False, stop=True)
                        else:
                            for hi in range(HT):
                                nc.tensor.matmul(
                                    ps[:], wblk(grp, t, hi),
                                    sp_t[:, hi * SC + c0:hi * SC + c0 + cw],
                                    start=(hi == 0), stop=(hi == HT - 1))
                        pdst = proj_sb[:, t * SC + c0:t * SC + c0 + cw]
                        if pc_cnt[0] < _cfg("act_pc", core):
                            nc.scalar.activation(pdst, ps[:], ident)
                        else:
                            nc.vector.tensor_copy(pdst, ps[:])
                        pc_cnt[0] += 1
                    # PE indicator-build of the built blocks, h-tile t
                    if lay.built:
                        bd = ps_bd.tile([128, lay.barea], f32, tag="bd")
                        c = 0
                        while c < lay.barea:
                            cw = min(512, lay.barea - c)
                            nc.tensor.matmul(
                                bd[:, c:c + cw],
                                projT_sb[:, t * 128:(t + 1) * 128],
                                ind_t[:, c:c + cw],
                                start=True, stop=True)
                            c += cw
                        hsec = H_t[:, t * S + lay.S_nb:(t + 1) * S]
                        nc.scalar.activation(hsec, bd[:], tanh,
                                             bias=blbr_t[:, t:t + 1])
                    # broadcast adds for non-built blocks, h-tile t (DVE)
                    for b in lay.nonbuilt:
                        h0 = t * S + b.boff
                        out_v = (H_t[:, h0:h0 + b.area]
                                 .rearrange("p (a w2 two) -> p a w2 two",
                                            a=b.a, two=2))
                        d0 = t * SC + lay.dense_off(b)
                        in0 = (proj_sb[:, d0:d0 + b.wpad]
                               .rearrange("p (w2 two) -> p w2 two", two=2)
                               .unsqueeze(1)
                               .broadcast_to([128, b.a, b.wpad // 2, 2]))
                        c0 = t * SC + lay.col_off(b)
                        in1 = (proj_sb[:, c0:c0 + 2 * b.a]
                               .rearrange("p (a two) -> p a two", two=2)
                               .unsqueeze(2)
                               .broadcast_to([128, b.a, b.wpad // 2, 2]))
                        nc.vector.tensor_tensor(out_v, in0, in1, op=add)
                    # tanh (bias free via ACT when unmerged, else in proj)
                    if MERGE == 1:
                        sec = H_t[:, t * S:t * S + lay.S_nb]
                        nc.scalar.activation(sec, sec, tanh,
                                             bias=blbr_t[:, t:t + 1])
                    elif (t + 1) % MERGE == 0:
                        assert not lay.built
                        sec = H_t[:, (t + 1 - MERGE) * S:(t + 1) * S]
                        nc.scalar.activation(sec, sec, tanh)
                    # previous rep's scores, interleaved at h-tile cadence
                    if prev[0] is not None and _cfg("sc_mode", core) != "end":
                        pend += emit_score_mms(prev[0], SLOT_GROUPS[t])
                        if _cfg("sc_mode", core) == "post":
                            emit_score_copies(out_sb, pend)
                            pend = []
                if prev[0] is not None:
                    if _cfg("sc_mode", core) == "end":
                        pend = emit_score_mms(prev[0], list(range(NG)))
                    if pend:
                        emit_score_copies(out_sb, pend)
                prev[0] = (H_t, wout_t,
                           wout32_t if PACK else None)

            out_sb = opool.tile([96, lay.PW] if PACK else [OUT, S],
                                f16, tag="osb")
            pend = emit_score_mms(prev[0], list(range(NG)))
            emit_score_copies(out_sb, pend)

    nc.compile()
    return nc


def _get_compiled(core):
    if core not in _COMPILED:
        _COMPILED[core] = _build_program(core)
    return _COMPILED[core]


# ---------------------------------------------------------------------------
# Host-side packing / scatter
# ---------------------------------------------------------------------------

def make_inputs(span_rep, Wl, bl, Wr, br, Wout, bout):
    span_rep = np.ascontiguousarray(np.asarray(span_rep, np.float32))
    Wl = np.asarray(Wl, np.float32)
    Wr = np.asarray(Wr, np.float32)
    Wout = np.asarray(Wout, np.float32)
    blbr = np.asarray(bl, np.float32) + np.asarray(br, np.float32)

    # wp: per (kind, hin hi, hout t) 128x128 blocks; kind 0 = Wl, 1 = Wr
    wp = np.empty((128, 2 * HT * HID), np.float16)
    for t in range(HT):
        for kind, M in ((0, Wl), (1, Wr)):
            for hi in range(HT):
                c0 = (kind * HT + hi) * HID + t * 128
                wp[:, c0:c0 + 128] = \
                    M[hi * 128:(hi + 1) * 128, t * 128:(t + 1) * 128]
    wout_p = np.empty((128, HT * OUT), np.float16)
    for t in range(HT):
        wout_p[:, OUT * t:OUT * (t + 1)] = Wout[t * 128:(t + 1) * 128]
    wout32 = np.zeros((128, HT * 32), np.float16)
    for t in range(HT):
        wout32[:, 32 * t:32 * t + OUT] = Wout[t * 128:(t + 1) * 128]
    blbr_p = np.empty((128, HT), np.float32)
    for t in range(HT):
        blbr_p[:, t] = blbr[t * 128:(t + 1) * 128]

    in_maps = []
    for core in range(NCORES):
        lay = LAYOUTS[core]
        spc = np.zeros((HID, lay.SPANCOLS), np.float32)
        for b in lay.nonbuilt:
            j = b.j
            left = span_rep[0:j, j].T          # [512, j]  (L spans)
            right = span_rep[j, j + 1:65].T    # [512, 64-j]  (R spans)
            dense, col = (left, right) if b.trans else (right, left)
            d0 = lay.dense_off(b)
            spc[:, d0:d0 + b.w] = dense
            c0 = lay.col_off(b)
            spc[:, c0:c0 + 2 * b.a:2] = col
            spc[:, c0 + 1:c0 + 2 * b.a:2] = col
        sp = np.empty((128, HT * lay.SPANCOLS), np.float16)
        for hi in range(HT):
            sp[:, hi * lay.SPANCOLS:(hi + 1) * lay.SPANCOLS] = \
                spc[hi * 128:(hi + 1) * 128]
        im = {"wp": wp, "sp": sp, "wout": wout_p, "blbr": blbr_p,
              "wout32": wout32,
              "blbr16": blbr.astype(np.float16).reshape(1, HID)}
        if lay.built:
            # group spans [64 L-slots | 64 R-slots] per built block + the
            # 0/1 indicator matrix (2 ones per valid H column)
            spgf = np.zeros((HID, lay.g_rows), np.float32)
            ind = np.zeros((128, lay.barea), np.float16)
            for gb, b in enumerate(lay.built):
                j = b.j
                g0 = gb * 128
                spgf[:, g0:g0 + j] = span_rep[0:j, j].T            # L spans
                spgf[:, g0 + 64:g0 + 64 + 64 - j] = \
                    span_rep[j, j + 1:65].T                        # R spans
                boff_rel = b.boff - lay.S_nb
                for ai in range(b.a):
                    crow = (g0 + ai) if not b.trans else (g0 + 64 + ai)
                    for wi in range(b.w):
                        col = boff_rel + ai * b.wpad + wi
                        drow = (g0 + 64 + wi) if not b.trans \
                            else (g0 + wi)
                        ind[crow, col] = 1.0
                        ind[drow, col] = 1.0
            spg = np.empty((128, HT * lay.g_rows), np.float16)
            for hi in range(HT):
                spg[:, hi * lay.g_rows:(hi + 1) * lay.g_rows] = \
                    spgf[hi * 128:(hi + 1) * 128]
            im["spg"] = spg
            im["ind"] = ind
        in_maps.append(im)
    return in_maps


def scatter_outputs(core_outs, bout):
    bout = np.asarray(bout, np.float32)
    full = np.zeros((N1, N1, N1, OUT), np.float32)
    for core in range(NCORES):
        lay = LAYOUTS[core]
        oc = np.asarray(core_outs[core], np.float32)
        if oc.shape[0] == 96:          # partition-packed score layout
            sc = np.empty((OUT, lay.S), np.float32)
            for ci, (c0, cw) in enumerate(lay.schunks):
                gi, k = ci // 3, ci % 3
                sc[:, c0:c0 + cw] = \
                    oc[32 * k:32 * k + OUT,
                       lay.pg_off[gi]:lay.pg_off[gi] + cw]
            oc = sc
        for b in lay.blocks:
            j = b.j
            blk = oc[:, b.boff:b.boff + b.area].reshape(OUT, b.a, b.wpad)
            if b.trans:
                full[0:j, j, j + 1:65, :] = \
                    blk[:, 0:64 - j, 0:j].transpose(2, 1, 0) + bout
            else:
                full[0:j, j, j + 1:65, :] = \
                    blk[:, 0:j, 0:64 - j].transpose(1, 2, 0) + bout
    return full


def kernel(span_rep, Wl, bl, Wr, br, Wout, bout):
    from concourse.bass_utils import run_bass_kernel_spmd

    in_maps = make_inputs(span_rep, Wl, bl, Wr, br, Wout, bout)
    core_outs = []
    for core in range(NCORES):
        nc = _get_compiled(core)
        res = run_bass_kernel_spmd(nc, [in_maps[core]], core_ids=[0])
        core_outs.append(res.results[0]["out"])
    return scatter_outputs(core_outs, bout)


if __name__ == "__main__":
    for c, js in enumerate(CORE_JS):
        lay = LAYOUTS[c]
        print(f"core {c}: js={sorted(js)} S={lay.S} SPANCOLS={lay.SPANCOLS} "
              f"pchunks={len(lay.pchunks)} schunks={len(lay.schunks)}")
